# revision 6
# baseline (speedup 1.0000x reference)
"""Fused multi-head-size-1 attention kernel for Trainium2 (Bass/Tile).

Problem: out = softmax((x_q Wq^T + bq)(x_k Wk^T + bk)^T / sqrt(D)) (x_v Wv^T + bv)
Shapes: B=8, QL=KL=2048, D=1024, fp32 in/out.

Sharding: data-parallel over batch. Core i processes batch i end-to-end;
no collectives. Host pre-transposes x/W to contraction-major layout and
casts matmul operands to bf16 (PE runs bf16 at 2 rows/cycle vs fp32's 1/2;
all accumulation stays fp32 in PSUM).

Algebraic fold (saves the whole K projection, 2.15 GMAC/core = 14%):
  S = (Xq Wq^T + bq)(Xk Wk^T + bk)^T
    = Xq M Xk^T + (Xq Wq^T bk) 1^T + 1 (Xk t)^T + (bq.bk) 1 1^T,
  with M = Wq^T Wk and t = Wk^T bq (both host-precomputed from weights).
  The terms constant along k cancel under softmax, so only Xk t survives;
  adding t to every column of G^T = (Xq M)^T before the score matmul
  reproduces it exactly (same per-partition bias add the Q proj used).

DMA: every streamed block is packed host-side so each SBUF partition's
data is contiguous in DRAM (2-16KB lines instead of 1KB) — small packets
cap the HWDGE queues well below HBM bandwidth. Two HWDGE queues split the
load: scalar/ACT pulls Wv, M, xq; sync/SP pulls xv, xk, then out.

Per-core dataflow (everything resident in SBUF in bf16):
  phase 1: V[k',h] = xv @ Wv^T + bv (ones col appended for the softmax
           denominator), G^T[e,q] = M^T @ xq^T (+t), Xk^T DMA'd raw.
  phase 2: per q-block: S^T[k',q] = Xk G^T (PSUM, fp32), P^T = exp(S^T/32)
           (ScalarE, bf16 out), O[q,h] (+l) = P V_aug (PSUM, fp32),
           O = O * (1/l) + bv, DMA out.
"""

import numpy as np
import ml_dtypes

import concourse.bass as bass
import concourse.mybir as mybir
from concourse.bacc import Bacc
from concourse.tile import TileContext
from concourse.bass_utils import run_bass_kernel_spmd

B, QL, KL, D = 8, 2048, 2048, 1024
P = 128
NCORES = 8
DT = D // P          # 8 tiles along d/e/h
KT = KL // P         # 16 tiles along k'
XCH = 512            # x streaming chunk along s
QB = 512             # q block for the attention stage
F32 = mybir.dt.float32
BF16 = mybir.dt.bfloat16
SCALE = 1.0 / 32.0   # 1/sqrt(D)

# x streaming chunk schedules (start, width). xv's first chunks are small
# so the PE starts as soon as possible after the DMA preamble.
XV_CHUNKS = [(0, 128), (128, 384), (512, 512), (1024, 512), (1536, 512)]
XQ_CHUNKS = [(0, 512), (512, 512), (1024, 512), (1536, 512)]
XK_CHUNKS = XQ_CHUNKS

# AV free-dim chunking over V's 1025 columns (1024 h + ones column for l).
# The l-carrying chunk goes first so the reciprocal overlaps the other
# chunks' matmuls.
AV_CHUNKS = [(684, 1025), (0, 342), (342, 684)]
AV_MAXW = 342


def _chunk_ap(param, chunks, idx):
    """AP for host-packed chunk `idx`: [P, DT, cw] with per-partition
    contiguous DRAM lines (DT*cw elements)."""
    base = sum(cw for _, cw in chunks[:idx]) * D
    cw = chunks[idx][1]
    return bass.AP(
        tensor=param[:].tensor,
        offset=base,
        ap=[[DT * cw, P], [cw, DT], [1, cw]],
    )


def build_bass() -> bass.Bass:
    # Bacc (not bare Bass): its finalize() runs the pass pipeline that splits
    # multi-semaphore waits into event semaphores (TRN2 allows 1 wait/inst).
    nc = Bacc()

    # host-packed streams (see prepare_in_maps for the packing)
    xqf = nc.declare_dram_parameter("xqf", [D * QL], BF16, isOutput=False)
    xkf = nc.declare_dram_parameter("xkf", [D * KL], BF16, isOutput=False)
    xvf = nc.declare_dram_parameter("xvf", [D * KL], BF16, isOutput=False)
    mf = nc.declare_dram_parameter("mf", [D * D], BF16, isOutput=False)
    wvf = nc.declare_dram_parameter("wvf", [D * D], BF16, isOutput=False)
    tp = nc.declare_dram_parameter("tp", [P, DT], F32, isOutput=False)
    bv = nc.declare_dram_parameter("bv", [D], F32, isOutput=False)
    out = nc.declare_dram_parameter("out", [QL, D], F32, isOutput=True)

    WV_CHUNKS = [(0, 512), (512, 512)]  # hc blocks of Wv
    M_CHUNKS = [(0, D)]

    with TileContext(nc) as tc:
        with (
            tc.tile_pool(name="persist", bufs=1) as persist,
            tc.tile_pool(name="consts", bufs=1) as consts,
        ):
            xk_sb = persist.tile([P, DT, KL], BF16, tag="xk")    # Xk^T[e%128, et, k']
            v_sb = persist.tile([P, KT, D + 1], BF16, tag="v")   # V[k'%128, kt, h|1]
            gt_sb = persist.tile([P, DT, QL], BF16, tag="gt")    # G^T[e%128, et, q]

            tp_sb = consts.tile([P, DT], F32, tag="tp")
            bv_sb = consts.tile([P, D], F32, tag="bv")
            nc.scalar.dma_start(out=tp_sb[:], in_=tp[:])
            # broadcast bv across all partitions (stride-0 partition AP -> SWDGE)
            bv_bcast = bass.AP(tensor=bv[:].tensor, offset=0, ap=[[0, P], [1, D]])
            nc.gpsimd.dma_start(out=bv_sb[:], in_=bv_bcast)

            # ---------------- phase 1: projections ----------------
            with (
                tc.tile_pool(name="wpool", bufs=2) as wpool,
                tc.tile_pool(name="xpool", bufs=3) as xpool,
                tc.tile_pool(name="projp", bufs=4, space="PSUM") as projp,
            ):
                # V first: its opening accumulation group only needs ONE
                # 512-col half of Wv plus a small first x chunk, so the PE
                # starts sooner after the DMA preamble.
                # V: out[s-tile, h-chunk] = sum_dt xvT[d,s-tile]^T @ WvT[d,h-chunk]
                # + bv (broadcast over rows), fused into the PSUM->SBUF move.
                w = wpool.tile([P, DT, D], BF16, tag="w")
                for hc in range(D // 512):
                    nc.scalar.dma_start(
                        out=w[:, :, hc * 512:(hc + 1) * 512],
                        in_=_chunk_ap(wvf, WV_CHUNKS, hc),
                    )
                for ci, (c0, cw) in enumerate(XV_CHUNKS):
                    xc = xpool.tile([P, DT, XCH], BF16, tag="x")
                    nc.sync.dma_start(out=xc[:, :, :cw], in_=_chunk_ap(xvf, XV_CHUNKS, ci))
                    for st4 in range(cw // P):
                        st = c0 // P + st4
                        for hc in range(D // 512):
                            ps = projp.tile([P, 512], F32, tag="proj")
                            for dt in range(DT):
                                nc.tensor.matmul(
                                    ps[:],
                                    lhsT=xc[:, dt, st4 * P:(st4 + 1) * P],
                                    rhs=w[:, dt, hc * 512:(hc + 1) * 512],
                                    start=(dt == 0),
                                    stop=(dt == DT - 1),
                                )
                            nc.any.tensor_add(
                                out=v_sb[:, st, hc * 512:(hc + 1) * 512],
                                in0=ps[:],
                                in1=bv_sb[:, hc * 512:(hc + 1) * 512],
                            )
                nc.vector.memset(v_sb[:, :, D], 1.0)  # ones column -> row sums

                # Xk^T raw load: on the sync HWDGE queue AFTER the xv chunks,
                # so FIFO order keeps it off the wire during the DMA-critical
                # preamble (gpsimd would be SWDGE: software DMA, which
                # measurably slows the whole kernel).
                for ci in range(len(XK_CHUNKS)):
                    c0, cw = XK_CHUNKS[ci]
                    nc.sync.dma_start(
                        out=xk_sb[:, :, c0:c0 + cw],
                        in_=_chunk_ap(xkf, XK_CHUNKS, ci),
                    )

                # G^T: out[e-tile, q-chunk] = sum_dt M[d,e-tile]^T @ xqT[d,q-chunk]
                # (+t per-partition). M and xq ride the scalar queue.
                w = wpool.tile([P, DT, D], BF16, tag="w")
                nc.scalar.dma_start(out=w[:], in_=_chunk_ap(mf, M_CHUNKS, 0))
                for cc in range(QL // XCH):
                    xc = xpool.tile([P, DT, XCH], BF16, tag="x")
                    nc.scalar.dma_start(out=xc[:], in_=_chunk_ap(xqf, XQ_CHUNKS, cc))
                    for ht in range(DT):
                        ps = projp.tile([P, XCH], F32, tag="proj")
                        for dt in range(DT):
                            nc.tensor.matmul(
                                ps[:],
                                lhsT=w[:, dt, ht * P:(ht + 1) * P],
                                rhs=xc[:, dt, :],
                                start=(dt == 0),
                                stop=(dt == DT - 1),
                            )
                        nc.any.tensor_scalar_add(
                            out=gt_sb[:, ht, cc * XCH:(cc + 1) * XCH],
                            in0=ps[:],
                            scalar1=tp_sb[:, ht:ht + 1],
                        )

            # ---------------- phase 2: attention ----------------
            with (
                tc.tile_pool(name="ptpool", bufs=2) as ptpool,
                tc.tile_pool(name="opool", bufs=3) as opool,
                tc.tile_pool(name="small", bufs=4) as small,
                tc.tile_pool(name="scorep", bufs=2, space="PSUM") as scorep,
                tc.tile_pool(name="avp", bufs=4, space="PSUM") as avp,
            ):
                for qb in range(QL // QB):
                    q0 = qb * QB
                    ptb = ptpool.tile([P, KT, QB], BF16, tag="pt")
                    # scores S^T[k', q] for two k'-tiles at a time
                    for kp in range(KT // 2):
                        sp = scorep.tile([P, 2 * QB], F32, tag="score")
                        for half in range(2):
                            kt = kp * 2 + half
                            for et in range(DT):
                                nc.tensor.matmul(
                                    sp[:, half * QB:(half + 1) * QB],
                                    lhsT=xk_sb[:, et, kt * P:(kt + 1) * P],
                                    rhs=gt_sb[:, et, q0:q0 + QB],
                                    start=(et == 0),
                                    stop=(et == DT - 1),
                                )
                        nc.scalar.activation(
                            out=ptb[:, kp * 2:(kp + 1) * 2, :].rearrange("p a b -> p (a b)"),
                            in_=sp[:],
                            func=mybir.ActivationFunctionType.Exp,
                            scale=SCALE,
                        )
                    # AV + row sums + normalize, one q-tile (128 rows) at a time.
                    for qt4 in range(QB // P):
                        qrow = q0 + qt4 * P
                        rl = small.tile([P, 1], F32, tag="rl")
                        ob = opool.tile([P, D], F32, tag="o")
                        for ci, (h0, h1) in enumerate(AV_CHUNKS):
                            av = avp.tile([P, AV_MAXW], F32, tag="av")
                            for kt in range(KT):
                                nc.tensor.matmul(
                                    av[:, :h1 - h0],
                                    lhsT=ptb[:, kt, qt4 * P:(qt4 + 1) * P],
                                    rhs=v_sb[:, kt, h0:h1],
                                    start=(kt == 0),
                                    stop=(kt == KT - 1),
                                )
                            if ci == 0:
                                # l (row sums) is the last column (global idx D)
                                nc.vector.reciprocal(rl[:], av[:, D - h0:D - h0 + 1])
                            w_ = min(h1, D) - h0
                            nc.any.tensor_scalar_mul(
                                out=ob[:, h0:h0 + w_],
                                in0=av[:, :w_],
                                scalar1=rl[:],
                            )
                            if qb == QL // QB - 1 and qt4 == QB // P - 1:
                                # very last q-tile: stream the output per chunk
                                # so the final DMA isn't serialized behind all
                                # three normalizes (shaves the tail barrier)
                                nc.sync.dma_start(
                                    out=out[qrow:qrow + P, h0:h0 + w_],
                                    in_=ob[:, h0:h0 + w_],
                                )
                        if not (qb == QL // QB - 1 and qt4 == QB // P - 1):
                            nc.sync.dma_start(out=out[qrow:qrow + P, :], in_=ob[:])

    nc.finalize()
    return nc


def _pack_chunks(xT, chunks):
    """[D, L] contraction-major -> flat concat of [P, DT, cw] blocks so each
    SBUF partition's chunk data is one contiguous DRAM line."""
    parts = []
    for c0, cw in chunks:
        blk = xT[:, c0:c0 + cw].reshape(DT, P, cw).transpose(1, 0, 2)
        parts.append(blk.reshape(-1))
    return np.ascontiguousarray(np.concatenate(parts))


def prepare_in_maps(q_embd, k_embd, v_embd, Wq, bq, Wk, bk, Wv, bv):
    bf16 = ml_dtypes.bfloat16
    f32 = np.float32

    def t_cast(x):  # [B, L, D] -> [B, D, L] bf16
        return np.ascontiguousarray(np.swapaxes(np.asarray(x, f32), 1, 2)).astype(bf16)

    xqT = t_cast(q_embd)
    xkT = t_cast(k_embd)
    xvT = t_cast(v_embd)
    # weight fold: S = Xq (Wq^T Wk) Xk^T + 1 (Xk Wk^T bq)^T  (+ terms that
    # cancel under softmax). M and t are weight-only precomputes.
    Wq_ = np.asarray(Wq, f32)
    Wk_ = np.asarray(Wk, f32)
    mT = np.ascontiguousarray(Wq_.T @ Wk_).astype(bf16)
    t_vec = Wk_.T @ np.asarray(bq, f32)
    tp = np.ascontiguousarray(t_vec.reshape(DT, P).T)
    wvT = np.ascontiguousarray(np.asarray(Wv, f32).T).astype(bf16)
    bv_ = np.ascontiguousarray(np.asarray(bv, f32))

    mf = _pack_chunks(mT, [(0, D)])
    wvf = _pack_chunks(wvT, [(0, 512), (512, 512)])

    return [
        {
            "xqf": _pack_chunks(xqT[i], XQ_CHUNKS),
            "xkf": _pack_chunks(xkT[i], XK_CHUNKS),
            "xvf": _pack_chunks(xvT[i], XV_CHUNKS),
            "mf": mf, "wvf": wvf, "tp": tp, "bv": bv_,
        }
        for i in range(NCORES)
    ]


_NC_CACHE = None


def get_nc() -> bass.Bass:
    global _NC_CACHE
    if _NC_CACHE is None:
        _NC_CACHE = build_bass()
    return _NC_CACHE


def run_on_device(in_maps, trace=False, **kwargs):
    return run_bass_kernel_spmd(get_nc(), in_maps, list(range(NCORES)), trace=trace, **kwargs)


def kernel(q_embd, k_embd, v_embd, Wq, bq, Wk, bk, Wv, bv):
    in_maps = prepare_in_maps(q_embd, k_embd, v_embd, Wq, bq, Wk, bk, Wv, bv)
    res = run_on_device(in_maps)
    return np.stack([r["out"] for r in res.results], axis=0)


# revision 7
# speedup vs baseline: 1.2159x; 1.2159x over previous
"""Fused multi-head-size-1 attention kernel for Trainium2 (Bass/Tile).

Problem: out = softmax((x_q Wq^T + bq)(x_k Wk^T + bk)^T / sqrt(D)) (x_v Wv^T + bv)
Shapes: B=8, QL=KL=2048, D=1024, fp32 in/out.

Sharding: data-parallel over batch. Core i processes batch i end-to-end;
no collectives. Host pre-transposes x/W to contraction-major layout and
casts matmul operands to bf16 (PE runs bf16 at 2 rows/cycle vs fp32's 1/2;
all accumulation stays fp32 in PSUM).

Algebraic fold (saves the whole K projection, 2.15 GMAC/core = 14%):
  S = (Xq Wq^T + bq)(Xk Wk^T + bk)^T
    = Xq M Xk^T + (Xq Wq^T bk) 1^T + 1 (Xk t)^T + (bq.bk) 1 1^T,
  with M = Wq^T Wk and t = Wk^T bq (both host-precomputed from weights).
  The terms constant along k cancel under softmax, so only Xk t survives;
  adding t to every column of G^T = (Xq M)^T before the score matmul
  reproduces it exactly (same per-partition bias add the Q proj used).

All DMAs stay on the two HWDGE queues with ~1KB lines: gpsimd/SWDGE or
8-16KB burst packets measurably flip the chip into a persistently ~1.2x
slower PE state (power throttle), costing far more than the DMA win.

Per-core dataflow (everything resident in SBUF in bf16):
  phase 1: V[k',h] = xv @ Wv^T + bv (ones col appended for the softmax
           denominator; hc=0 groups for all chunks first so the PE only
           needs the first Wv half during the DMA preamble),
           G^T[e,q] = M^T @ xq^T (+t), Xk^T DMA'd raw between them.
  phase 2: per q-block: S^T[k',q] = Xk G^T (PSUM, fp32), P^T = exp(S^T/32)
           (ScalarE, bf16 out), O[q,h] (+l) = P V_aug (PSUM, fp32),
           O = O * (1/l) + bv, DMA out.
"""

import numpy as np
import ml_dtypes

import concourse.bass as bass
import concourse.mybir as mybir
from concourse.bacc import Bacc
from concourse.tile import TileContext
from concourse.bass_utils import run_bass_kernel_spmd

B, QL, KL, D = 8, 2048, 2048, 1024
P = 128
NCORES = 8
DT = D // P          # 8 tiles along d/e/h
KT = KL // P         # 16 tiles along k'
XCH = 512            # x streaming chunk along s
QB = 512             # q block for the attention stage
F32 = mybir.dt.float32
BF16 = mybir.dt.bfloat16
SCALE = 1.0 / 32.0   # 1/sqrt(D)

XV_CHUNKS = [(0, 128), (128, 384), (512, 512), (1024, 512), (1536, 512)]

# AV free-dim chunking over V's 1025 columns (1024 h + ones column for l).
# The l-carrying chunk goes first so the reciprocal overlaps the other
# chunks' matmuls.
AV_CHUNKS = [(684, 1025), (0, 342), (342, 684)]
AV_MAXW = 342


def build_bass() -> bass.Bass:
    # Bacc (not bare Bass): its finalize() runs the pass pipeline that splits
    # multi-semaphore waits into event semaphores (TRN2 allows 1 wait/inst).
    nc = Bacc()

    xqT = nc.declare_dram_parameter("xqT", [D, QL], BF16, isOutput=False)
    xkT = nc.declare_dram_parameter("xkT", [D, KL], BF16, isOutput=False)
    xvT = nc.declare_dram_parameter("xvT", [D, KL], BF16, isOutput=False)
    mT = nc.declare_dram_parameter("mT", [D, D], BF16, isOutput=False)
    wvT = nc.declare_dram_parameter("wvT", [D, D], BF16, isOutput=False)
    tp = nc.declare_dram_parameter("tp", [P, DT], F32, isOutput=False)
    bv = nc.declare_dram_parameter("bv", [D], F32, isOutput=False)
    out = nc.declare_dram_parameter("out", [QL, D], F32, isOutput=True)

    # contraction-major views: d = dt*128 + p
    xq_r = xqT[:].rearrange("(dt p) s -> p dt s", p=P)
    xk_r = xkT[:].rearrange("(dt p) s -> p dt s", p=P)
    xv_r = xvT[:].rearrange("(dt p) s -> p dt s", p=P)
    m_r = mT[:].rearrange("(dt p) h -> p dt h", p=P)
    wv_r = wvT[:].rearrange("(dt p) h -> p dt h", p=P)

    with TileContext(nc) as tc:
        with (
            tc.tile_pool(name="persist", bufs=1) as persist,
            tc.tile_pool(name="consts", bufs=1) as consts,
        ):
            xk_sb = persist.tile([P, DT, KL], BF16, tag="xk")    # Xk^T[e%128, et, k']
            v_sb = persist.tile([P, KT, D + 1], BF16, tag="v")   # V[k'%128, kt, h|1]
            gt_sb = persist.tile([P, DT, QL], BF16, tag="gt")    # G^T[e%128, et, q]

            tp_sb = consts.tile([P, DT], F32, tag="tp")
            bv_sb = consts.tile([P, D], F32, tag="bv")
            nc.scalar.dma_start(out=tp_sb[:], in_=tp[:])
            # broadcast bv across all partitions (stride-0 partition AP -> SWDGE)
            bv_bcast = bass.AP(tensor=bv[:].tensor, offset=0, ap=[[0, P], [1, D]])
            nc.gpsimd.dma_start(out=bv_sb[:], in_=bv_bcast)

            # ---------------- phase 1: projections ----------------
            with (
                tc.tile_pool(name="wpool", bufs=2) as wpool,
                tc.tile_pool(name="xpool", bufs=6) as xpool,
                tc.tile_pool(name="projp", bufs=4, space="PSUM") as projp,
            ):
                # V: out[s-tile, h-chunk] = sum_dt xvT[d,s-tile]^T @ WvT[d,h-chunk]
                # + bv (broadcast over rows), fused into the PSUM->SBUF move.
                # Two passes over the x chunks (hc=0 for all, then hc=1): the
                # PE's first ~27us only needs the first 1MB half of Wv, so the
                # preamble DMA burden is halved.
                w = wpool.tile([P, DT, D], BF16, tag="w")
                for hc in range(D // 512):
                    for dt in range(DT):
                        nc.scalar.dma_start(
                            out=w[:, dt, hc * 512:(hc + 1) * 512],
                            in_=wv_r[:, dt, hc * 512:(hc + 1) * 512],
                        )
                xcs = []
                for ci, (c0, cw) in enumerate(XV_CHUNKS):
                    xc = xpool.tile([P, DT, XCH], BF16, tag="x")
                    xcs.append(xc)
                    nc.sync.dma_start(out=xc[:, :, :cw], in_=xv_r[:, :, c0:c0 + cw])
                    for st4 in range(cw // P):
                        st = c0 // P + st4
                        ps = projp.tile([P, 512], F32, tag="proj")
                        for dt in range(DT):
                            nc.tensor.matmul(
                                ps[:],
                                lhsT=xc[:, dt, st4 * P:(st4 + 1) * P],
                                rhs=w[:, dt, 0:512],
                                start=(dt == 0),
                                stop=(dt == DT - 1),
                            )
                        nc.any.tensor_add(
                            out=v_sb[:, st, 0:512],
                            in0=ps[:],
                            in1=bv_sb[:, 0:512],
                        )
                for ci, (c0, cw) in enumerate(XV_CHUNKS):
                    xc = xcs[ci]
                    for st4 in range(cw // P):
                        st = c0 // P + st4
                        ps = projp.tile([P, 512], F32, tag="proj")
                        for dt in range(DT):
                            nc.tensor.matmul(
                                ps[:],
                                lhsT=xc[:, dt, st4 * P:(st4 + 1) * P],
                                rhs=w[:, dt, 512:1024],
                                start=(dt == 0),
                                stop=(dt == DT - 1),
                            )
                        nc.any.tensor_add(
                            out=v_sb[:, st, 512:1024],
                            in0=ps[:],
                            in1=bv_sb[:, 512:1024],
                        )
                nc.vector.memset(v_sb[:, :, D], 1.0)  # ones column -> row sums

                # Xk^T raw load: emitted on the sync HWDGE queue AFTER the xv
                # chunks, so FIFO order keeps it off the wire during the
                # DMA-critical preamble.
                for cc in range(KL // XCH):
                    nc.sync.dma_start(
                        out=xk_sb[:, :, cc * XCH:(cc + 1) * XCH],
                        in_=xk_r[:, :, cc * XCH:(cc + 1) * XCH],
                    )

                # G^T: out[e-tile, q-chunk] = sum_dt M[d,e-tile]^T @ xqT[d,q-chunk]
                # (+t per-partition).
                w = wpool.tile([P, DT, D], BF16, tag="w")
                nc.scalar.dma_start(out=w[:], in_=m_r)
                for cc in range(QL // XCH):
                    xc = xpool.tile([P, DT, XCH], BF16, tag="x")
                    nc.sync.dma_start(out=xc[:], in_=xq_r[:, :, cc * XCH:(cc + 1) * XCH])
                    for ht in range(DT):
                        ps = projp.tile([P, XCH], F32, tag="proj")
                        for dt in range(DT):
                            nc.tensor.matmul(
                                ps[:],
                                lhsT=w[:, dt, ht * P:(ht + 1) * P],
                                rhs=xc[:, dt, :],
                                start=(dt == 0),
                                stop=(dt == DT - 1),
                            )
                        nc.any.tensor_scalar_add(
                            out=gt_sb[:, ht, cc * XCH:(cc + 1) * XCH],
                            in0=ps[:],
                            scalar1=tp_sb[:, ht:ht + 1],
                        )

            # ---------------- phase 2: attention ----------------
            with (
                tc.tile_pool(name="ptpool", bufs=2) as ptpool,
                tc.tile_pool(name="opool", bufs=3) as opool,
                tc.tile_pool(name="small", bufs=4) as small,
                tc.tile_pool(name="scorep", bufs=2, space="PSUM") as scorep,
                tc.tile_pool(name="avp", bufs=4, space="PSUM") as avp,
            ):
                for qb in range(QL // QB):
                    q0 = qb * QB
                    ptb = ptpool.tile([P, KT, QB], BF16, tag="pt")
                    # scores S^T[k', q] for two k'-tiles at a time
                    for kp in range(KT // 2):
                        sp = scorep.tile([P, 2 * QB], F32, tag="score")
                        for half in range(2):
                            kt = kp * 2 + half
                            for et in range(DT):
                                nc.tensor.matmul(
                                    sp[:, half * QB:(half + 1) * QB],
                                    lhsT=xk_sb[:, et, kt * P:(kt + 1) * P],
                                    rhs=gt_sb[:, et, q0:q0 + QB],
                                    start=(et == 0),
                                    stop=(et == DT - 1),
                                )
                        nc.scalar.activation(
                            out=ptb[:, kp * 2:(kp + 1) * 2, :].rearrange("p a b -> p (a b)"),
                            in_=sp[:],
                            func=mybir.ActivationFunctionType.Exp,
                            scale=SCALE,
                        )
                    # AV + row sums + normalize, one q-tile (128 rows) at a time.
                    for qt4 in range(QB // P):
                        qrow = q0 + qt4 * P
                        rl = small.tile([P, 1], F32, tag="rl")
                        ob = opool.tile([P, D], F32, tag="o")
                        for ci, (h0, h1) in enumerate(AV_CHUNKS):
                            av = avp.tile([P, AV_MAXW], F32, tag="av")
                            for kt in range(KT):
                                nc.tensor.matmul(
                                    av[:, :h1 - h0],
                                    lhsT=ptb[:, kt, qt4 * P:(qt4 + 1) * P],
                                    rhs=v_sb[:, kt, h0:h1],
                                    start=(kt == 0),
                                    stop=(kt == KT - 1),
                                )
                            if ci == 0:
                                # l (row sums) is the last column (global idx D)
                                nc.vector.reciprocal(rl[:], av[:, D - h0:D - h0 + 1])
                            w_ = min(h1, D) - h0
                            nc.any.tensor_scalar_mul(
                                out=ob[:, h0:h0 + w_],
                                in0=av[:, :w_],
                                scalar1=rl[:],
                            )
                            if qb == QL // QB - 1 and qt4 == QB // P - 1:
                                # very last q-tile: stream the output per chunk
                                # so the final DMA isn't serialized behind all
                                # three normalizes (shaves the tail barrier)
                                nc.sync.dma_start(
                                    out=out[qrow:qrow + P, h0:h0 + w_],
                                    in_=ob[:, h0:h0 + w_],
                                )
                        if not (qb == QL // QB - 1 and qt4 == QB // P - 1):
                            nc.sync.dma_start(out=out[qrow:qrow + P, :], in_=ob[:])

    nc.finalize()
    return nc


def prepare_in_maps(q_embd, k_embd, v_embd, Wq, bq, Wk, bk, Wv, bv):
    bf16 = ml_dtypes.bfloat16
    f32 = np.float32

    def t_cast(x):  # [B, L, D] -> [B, D, L] bf16
        return np.ascontiguousarray(np.swapaxes(np.asarray(x, f32), 1, 2)).astype(bf16)

    xqT = t_cast(q_embd)
    xkT = t_cast(k_embd)
    xvT = t_cast(v_embd)
    # weight fold: S = Xq (Wq^T Wk) Xk^T + 1 (Xk Wk^T bq)^T  (+ terms that
    # cancel under softmax). M and t are weight-only precomputes.
    Wq_ = np.asarray(Wq, f32)
    Wk_ = np.asarray(Wk, f32)
    mT = np.ascontiguousarray(Wq_.T @ Wk_).astype(bf16)
    t_vec = Wk_.T @ np.asarray(bq, f32)
    tp = np.ascontiguousarray(t_vec.reshape(DT, P).T)
    wvT = np.ascontiguousarray(np.asarray(Wv, f32).T).astype(bf16)
    bv_ = np.ascontiguousarray(np.asarray(bv, f32))

    return [
        {
            "xqT": xqT[i], "xkT": xkT[i], "xvT": xvT[i],
            "mT": mT, "wvT": wvT, "tp": tp, "bv": bv_,
        }
        for i in range(NCORES)
    ]


_NC_CACHE = None


def get_nc() -> bass.Bass:
    global _NC_CACHE
    if _NC_CACHE is None:
        _NC_CACHE = build_bass()
    return _NC_CACHE


def run_on_device(in_maps, trace=False, **kwargs):
    return run_bass_kernel_spmd(get_nc(), in_maps, list(range(NCORES)), trace=trace, **kwargs)


def kernel(q_embd, k_embd, v_embd, Wq, bq, Wk, bk, Wv, bv):
    in_maps = prepare_in_maps(q_embd, k_embd, v_embd, Wq, bq, Wk, bk, Wv, bv)
    res = run_on_device(in_maps)
    return np.stack([r["out"] for r in res.results], axis=0)
